# revision 1
# baseline (speedup 1.0000x reference)
"""Trainium2 Bass kernel for CodebookConv1D (VQ-dequant + GPT2-Conv1D matmul).

Computation: W = codebook[indices].reshape(2048, 8192); out = x @ W + bias.
Sharding: tensor-parallel over out_features (8192 -> 8 cores x 1024 columns).

Per core (out columns split into 8 n-chunks of 128):
  - W shard (2048 x 1024 bf16) lives in SBUF in [k_part, g, kc, ncol]
    layout. n-chunks g=5..7 are gathered ON DEVICE from a host-padded
    bf16 codebook (4096 x 128 bf16 = 256B row stride) with
    InstDMAGatherAnt: 16B payload per descriptor, 1920 descriptors per
    instruction, round-robin over the 4 SWDGE queues (the 4 queues
    generate descriptors in parallel at ~8ns/descriptor each;
    single_packet must be False for >16KB payloads, and >>1000
    descriptors per instruction overflows the shared ring carveout
    under concurrent DMA traffic). n-chunks g=0..4 are dequantized on
    the host and DMA'd in directly -- the device gather of g=5..7 runs
    concurrently and is fully hidden behind the matmuls on g=0..4.
  - Matmul is W-stationary: lhsT = W chunk [128k x 128n], moving = x
    tile [128k x 512m] bf16 (single-PSUM-bank cap), accumulating over
    16 k-chunks; two interleaved PSUM banks cover a 1024-row m-chunk.
  - The (g, mc) grid is walked in 2 windows of 4 resident x m-chunks
    (g-inner) so the PE keeps working on host-supplied chunks while
    the device gather streams in the rest.
  - Bias is added on the Activation engine (per-partition bias vector)
    while copying PSUM -> SBUF; output is stored n-on-partitions
    (transposed), and the host transposes back when unsharding.
  - x is pre-transposed/cast to bf16 on the host in a tiled layout so
    each m-chunk load is a single contiguous-per-partition DMA.
"""

import sys

if "/opt/trn_rl_repo" not in sys.path:
    sys.path.insert(0, "/opt/trn_rl_repo")

import numpy as np

IN_F = 2048
OUT_F = 8192
K_CB = 4096
BLOCK = 8
N_CORES = 8
M_FULL = 8192                      # 4*2048 tokens
N_PER = OUT_F // N_CORES           # 1024 out columns per core
KC = IN_F // 128                   # 16 k-chunks
NCH = N_PER // 128                 # 8 n-chunks of 128 columns per core
CB_PAD = 128                       # padded bf16 codebook row: 128*2B = 256B
COLS_PER_CH = 256                  # gather C-columns per n-chunk (128 idx each)
DEV_G = 2                          # n-chunks gathered on device (g >= NCH-DEV_G)
HOST_G = NCH - DEV_G               # n-chunks dequantized on host
DEV_COLS = DEV_G * COLS_PER_CH     # 768 device gather columns
COLS_PER_INST = 15                 # 1920 idxs / 121 ring descs per instruction
WINDOW = 8                         # resident 512-row x half-tiles per window

_CACHE = {}


def _emit_dma_gather(
    nc, mybir, out_ap, in_ap, idxs_ap, num_idxs, elem_size, elem_step, queue_num=0
):
    """InstDMAGatherAnt with a sub-256B payload (allowed for non-transpose;
    bass.dma_gather's %256 assert only applies to transpose mode). The
    256B-granularity constraint is on the source row stride (elem_step)."""
    eng = nc.gpsimd
    _in_ap = eng.lower_ap_dma(in_ap, for_custom_bir_dma=True)
    _idxs_ap = eng.lower_ap(idxs_ap)
    _out_ap = eng.lower_ap(out_ap)
    stride_bytes = elem_step * mybir.dt.size(in_ap.dtype)
    assert stride_bytes % 256 == 0
    return eng.add_instruction(
        mybir.InstDMAGatherAnt(
            name=nc.get_next_instruction_name(),
            ins=[*_in_ap, _idxs_ap, eng.lower_val_access(eng.to_reg(num_idxs))],
            outs=[_out_ap],
            transpose=False,
            num_idxs=num_idxs,
            elem_size=elem_size,
            stride_bytes_256=stride_bytes // 256,
            gen_mode=0,
            # single_packet=True caps the total gathered payload at 16KB
            # (NRT exec error beyond that); False lifts the cap.
            single_packet=False,
            queue_num=queue_num,
            sbuf_tokens_per_rank=0,
            sbuf_free_dim_per_rank=0,
            sbuf_free_dim_pad_per_rank=0,
            sbuf_byte_offset=0,
        )
    )


def _build(n_mchunks=M_FULL // 1024):
    import concourse.bacc as bacc
    import concourse.mybir as mybir
    import concourse.tile as tile
    from concourse.library_config import mlp

    f32 = mybir.dt.float32
    bf16 = mybir.dt.bfloat16
    i16 = mybir.dt.int16
    m_rows = n_mchunks * 1024

    nc = bacc.Bacc("TRN2", target_bir_lowering=False, num_swdge_queues=4)
    # Host-tiled x: xtt[p, mh, kc, m] = x[mh*512+m, kc*128+p], bf16
    n_mh_glob = 2 * n_mchunks
    xtt_d = nc.dram_tensor("xtt", [128, n_mh_glob, KC, 512], bf16,
                           kind="ExternalInput")
    # Padded codebook: row stride 256B, payload = first 8 bf16 of each row
    cbp_d = nc.dram_tensor("cbp", [K_CB, CB_PAD], bf16, kind="ExternalInput")
    # Wrapped gather indices for device n-chunks (int16, 16-partition wrap)
    idx_d = nc.dram_tensor("idx", [128, DEV_COLS * 8], i16,
                           kind="ExternalInput")
    # Host-dequantized W for n-chunks 0..HOST_G-1, in w_all layout
    wh_d = nc.dram_tensor("wh", [128, HOST_G * COLS_PER_CH * BLOCK], bf16,
                          kind="ExternalInput")
    # biasT[p, g] = bias[g*128 + p]
    bias_d = nc.dram_tensor("biasT", [128, NCH], f32, kind="ExternalInput")
    # Output stored transposed: outT[n, m]
    out_d = nc.dram_tensor("outT", [N_PER, m_rows], f32, kind="ExternalOutput")

    with tile.TileContext(nc) as tc:
        with (
            tc.tile_pool(name="const", bufs=1) as constp,
            tc.tile_pool(name="wpool", bufs=1) as wpool,
            tc.tile_pool(name="xio", bufs=WINDOW) as xio,
            tc.tile_pool(name="outp", bufs=3) as outp,
            tc.tile_pool(name="psum", bufs=8, space="PSUM") as psump,
        ):
            nc.gpsimd.load_library(mlp)

            # Constants: host W chunks + bias on the scalar queue. The
            # gather indices go on the sync queue BEHIND window 0's x
            # loads: the gather's flood of 16B descriptors congests the
            # DMA engines, so it must not overlap the startup bulk loads.
            idx_t = constp.tile([128, DEV_COLS * 8], i16)
            bias_t = constp.tile([128, NCH], f32)
            # W shard, bf16; free offset of (g, kc) chunk = (g*256+kc*16)*8
            w_all = wpool.tile([128, NCH * COLS_PER_CH * BLOCK], bf16)
            ch_elems = COLS_PER_CH * BLOCK
            def _wh_load(g):
                nc.sync.dma_start(
                    out=w_all[:, g * ch_elems : (g + 1) * ch_elems],
                    in_=wh_d[:, g * ch_elems : (g + 1) * ch_elems],
                )

            # Single ring (sync), strictly in demand order: the DMA engines
            # drain a ring's descriptors in order, so the first matmul's
            # inputs (wh0 + xb0) get the full HBM bandwidth.
            n_mh = 2 * n_mchunks
            w0_mhs = list(range(min(WINDOW, n_mh)))
            xbs0 = []

            def _xb_load(mh):
                xb = xio.tile([128, KC, 512], bf16, tag="xb")
                nc.sync.dma_start(out=xb[:], in_=xtt_d[:, mh])
                xbs0.append(xb)

            _wh_load(0)
            _xb_load(w0_mhs[0])
            _wh_load(1)
            nc.sync.dma_start(out=bias_t[:], in_=bias_d[:, :])
            if len(w0_mhs) > 1:
                _xb_load(w0_mhs[1])
            for g in range(2, HOST_G):
                _wh_load(g)
            for mh in w0_mhs[2:4]:
                _xb_load(mh)
            nc.sync.dma_start(out=idx_t[:], in_=idx_d[:, :])
            for mh in w0_mhs[4:]:
                _xb_load(mh)

            qn = 0
            s = 0
            while s < DEV_COLS:
                n_c = min(COLS_PER_INST, DEV_COLS - s)
                c0 = HOST_G * COLS_PER_CH + s
                _emit_dma_gather(
                    nc,
                    mybir,
                    out_ap=w_all[:, c0 * BLOCK : (c0 + n_c) * BLOCK],
                    in_ap=cbp_d[:, 0:BLOCK],
                    idxs_ap=idx_t[:, s * 8 : (s + n_c) * 8],
                    num_idxs=n_c * 128,
                    elem_size=BLOCK,
                    elem_step=CB_PAD,
                    queue_num=qn % 4,
                )
                qn += 1
                s += n_c

            n_windows = (n_mh + WINDOW - 1) // WINDOW
            for win in range(n_windows):
                mhs = list(range(win * WINDOW, min((win + 1) * WINDOW, n_mh)))
                if win == 0:
                    xbs = xbs0
                    # host-supplied chunks mh-outer (PE only needs the first
                    # x half-tile + wh to start), then device-gathered
                    # chunks g-outer so the last gather has the longest
                    # deadline
                    order = [(j, g) for j in range(len(mhs))
                             for g in range(HOST_G)]
                    order += [(j, g) for g in range(HOST_G, NCH)
                              for j in range(len(mhs))]
                else:
                    xbs = []
                    for mh in mhs:
                        xb = xio.tile([128, KC, 512], bf16, tag="xb")
                        nc.sync.dma_start(out=xb[:], in_=xtt_d[:, mh])
                        xbs.append(xb)
                    order = [(j, g) for j in range(len(mhs))
                             for g in range(NCH)]
                for j, g in order:
                    mh = mhs[j]
                    ps = psump.tile([128, 512], f32, tag="ps")
                    for kc in range(KC):
                        off = (g * COLS_PER_CH + kc * 16) * BLOCK
                        nc.tensor.matmul(
                            out=ps[:],
                            lhsT=w_all[:, off : off + 128],
                            rhs=xbs[j][:, kc, :],
                            start=(kc == 0),
                            stop=(kc == KC - 1),
                        )
                    ot = outp.tile([128, 512], f32, tag="ot")
                    nc.scalar.add(
                        out=ot[:], in_=ps[:], add=bias_t[:, g : g + 1]
                    )
                    nc.scalar.dma_start(
                        out=out_d[g * 128 : (g + 1) * 128,
                                  mh * 512 : (mh + 1) * 512],
                        in_=ot[:],
                    )
    nc.compile()
    return nc


def get_nc(n_mchunks=M_FULL // 1024):
    key = ("nc", n_mchunks)
    if key not in _CACHE:
        _CACHE[key] = _build(n_mchunks)
    return _CACHE[key]


def make_in_maps(x, codebook, indices, bias, n_mchunks=M_FULL // 1024):
    """Host-side sharding: full inputs -> per-core input dicts."""
    import ml_dtypes

    bf16 = ml_dtypes.bfloat16
    m_rows = n_mchunks * 1024

    xm = np.asarray(x, dtype=np.float32).reshape(M_FULL, IN_F)[:m_rows]
    # xtt[p, mh, kc, m] = xm[mh*512+m, kc*128+p]
    xtt = np.ascontiguousarray(
        xm.reshape(2 * n_mchunks, 512, KC, 128).transpose(3, 0, 2, 1)
    ).astype(bf16)

    cbb = np.asarray(codebook, dtype=np.float32).astype(bf16)
    cbp = np.zeros((K_CB, CB_PAD), dtype=bf16)
    cbp[:, :BLOCK] = cbb

    idx_all = np.asarray(indices, dtype=np.int64).reshape(IN_F, OUT_F // BLOCK)
    bias = np.asarray(bias, dtype=np.float32)

    # Host dequant of the full W in block form: [row, blkcol, 8] bf16
    wb = cbb[idx_all]  # (2048, 1024, 8)

    in_maps = []
    nblk_per = N_PER // BLOCK  # 128 block-columns per core
    for c in range(N_CORES):
        sl = slice(c * nblk_per, (c + 1) * nblk_per)
        # a[kc, p, g, cb] = idx of block (row kc*128+p, blkcol g*16+cb)
        a = idx_all[:, sl].reshape(KC, 128, NCH, 16)
        # device chunks g >= HOST_G; gather order i = ((C-C0)*128 + p),
        # C = g*256 + kc*16 + cb
        flat = np.ascontiguousarray(
            a[:, :, HOST_G:, :].transpose(2, 0, 3, 1)
        ).reshape(-1).astype(np.int16)
        wrapped = np.ascontiguousarray(flat.reshape(-1, 16).T)
        idx_host = np.tile(wrapped, (8, 1))  # [128, DEV_COLS*8]

        # host W chunks g < HOST_G: wh[p, (g, kc, cb, j)]
        wc = wb[:, sl].reshape(KC, 128, NCH, 16, BLOCK)
        wh = np.ascontiguousarray(
            wc[:, :, :HOST_G].transpose(1, 2, 0, 3, 4)
        ).reshape(128, HOST_G * COLS_PER_CH * BLOCK)

        bias_c = np.ascontiguousarray(
            bias[c * N_PER : (c + 1) * N_PER].reshape(NCH, 128).T
        )
        in_maps.append(
            {
                "xtt": xtt,
                "cbp": cbp,
                "idx": idx_host,
                "wh": wh,
                "biasT": bias_c,
            }
        )
    return in_maps


def kernel(x, codebook, indices, bias):
    from concourse.bass_utils import run_bass_kernel_spmd

    nc = get_nc()
    in_maps = make_in_maps(x, codebook, indices, bias)
    res = run_bass_kernel_spmd(nc, in_maps, core_ids=list(range(N_CORES)))
    # outT is [n, m] per core; stack cores along n then transpose to [m, n]
    full = np.concatenate(
        [np.asarray(res.results[c]["outT"], dtype=np.float32)
         for c in range(N_CORES)],
        axis=0,
    )
    out = np.ascontiguousarray(full.T).reshape(4, 2048, OUT_F)
    return out.astype(np.float32, copy=False)



# revision 8
# speedup vs baseline: 1.1266x; 1.1266x over previous
"""Trainium2 Bass kernel for CodebookConv1D (VQ-dequant + GPT2-Conv1D matmul).

Computation: W = codebook[indices].reshape(2048, 8192); out = x @ W + bias.
Sharding: tensor-parallel over out_features (8192 -> 8 cores x 1024 columns).

Per core (out columns split into 8 n-chunks of 128, tokens into 16 m-halves
of 512), a mixed-precision split-K scheme:
  - K = 2048 is split 4 chunks fp8-e4m3 + 12 chunks bf16. The fp8 chunks
    run as fp8 DoubleRow matmuls (2 k-chunks per MM, ~250 ns vs 2x216 ns
    for bf16) which cuts PE time ~10.5% while the quantization error stays
    at rel ~0.0185 < 2e-2 (error dilutes as sqrt(K_fp8/K); pure fp8 would
    be 0.038).
  - DoubleRow MMs pay a ~135 ns mode-switch penalty when adjacent to bf16
    MMs, so the (mh) superblock opens all 8 n-groups' accumulations with
    16 back-to-back DoubleRow MMs (phase A, one PSUM bank per n-chunk g),
    then closes them with 96 back-to-back bf16 MMs (phase B): 2 switches
    per 24.7 us instead of per group.
  - W is fully dequantized on the host (cb[idx]) and shipped pre-cast:
    wb (bf16 12/16 of rows) + wf (e4m3 4/16, DoubleRow pair layout). x is
    host-transposed/cast the same way. Total HBM in+out ~65 MB/core,
    well under the PE time at 358 GB/s.
  - Bias is added on the Activation engine (per-partition bias vector)
    while copying PSUM -> SBUF; output is stored n-on-partitions
    (transposed), the host transposes back when unsharding.
"""

import sys

if "/opt/trn_rl_repo" not in sys.path:
    sys.path.insert(0, "/opt/trn_rl_repo")

import numpy as np

IN_F = 2048
OUT_F = 8192
K_CB = 4096
BLOCK = 8
N_CORES = 8
M_FULL = 8192                      # 4*2048 tokens
N_PER = OUT_F // N_CORES           # 1024 out columns per core
KC = IN_F // 128                   # 16 k-chunks
NCH = N_PER // 128                 # 8 n-chunks of 128 columns per core
NF8 = 4                            # k-chunks in fp8 (kc 0..3), as 2 DR pairs
NDR = NF8 // 2                     # DoubleRow matmuls per group
NB = KC - NF8                      # bf16 k-chunks (kc 4..15)
N_MH = M_FULL // 512               # 16 m-halves of 512 tokens
WINDOW = 8                         # resident m-halves per window

_CACHE = {}


def _build():
    import concourse.bacc as bacc
    import concourse.mybir as mybir
    import concourse.tile as tile

    f32 = mybir.dt.float32
    bf16 = mybir.dt.bfloat16
    f8e4 = mybir.dt.float8e4
    DR = mybir.MatmulPerfMode.DoubleRow

    nc = bacc.Bacc("TRN2", target_bir_lowering=False)
    # Host-tiled x, bf16 part: xtb[p, mh, kc, m] = x[mh*512+m, (NF8+kc)*128+p]
    xtb_d = nc.dram_tensor("xtb", [128, N_MH, NB, 512], bf16,
                           kind="ExternalInput")
    # fp8 part in DoubleRow pair layout: xtf[p, mh, pr, s, m]
    xtf_d = nc.dram_tensor("xtf", [128, N_MH, NDR, 2, 512], f8e4,
                           kind="ExternalInput")
    # W shards, host-dequantized: wb[p, (g, kc, col)], wf[p, (g, pr, s, col)]
    wb_d = nc.dram_tensor("wb", [128, NCH, NB * 128], bf16,
                          kind="ExternalInput")
    wf_d = nc.dram_tensor("wf", [128, NCH, NDR, 2, 128], f8e4,
                          kind="ExternalInput")
    # biasT[p, g] = bias[g*128 + p]
    bias_d = nc.dram_tensor("biasT", [128, NCH], f32, kind="ExternalInput")
    # Output stored transposed: outT[n, m]
    out_d = nc.dram_tensor("outT", [N_PER, M_FULL], f32, kind="ExternalOutput")

    with tile.TileContext(nc) as tc:
        with (
            tc.tile_pool(name="const", bufs=1) as constp,
            tc.tile_pool(name="xio", bufs=WINDOW) as xio,
            tc.tile_pool(name="outp", bufs=4) as outp,
            tc.tile_pool(name="psum", bufs=8, space="PSUM") as psump,
        ):
            bias_t = constp.tile([128, NCH], f32)
            wf_t = constp.tile([128, NCH, NDR, 2, 128], f8e4)
            wb_t = constp.tile([128, NCH, NB * 128], bf16)
            warm_t = constp.tile([128, 8], bf16)

            xbs, xfs = {}, {}

            def _x_load(mh):
                xf = xio.tile([128, NDR, 2, 512], f8e4, tag="xf")
                nc.sync.dma_start(out=xf[:], in_=xtf_d[:, mh])
                xfs[mh] = xf
                xb = xio.tile([128, NB, 512], bf16, tag="xb")
                nc.sync.dma_start(out=xb[:], in_=xtb_d[:, mh])
                xbs[mh] = xb

            # HAM warmup: ~3.5us of tiny matmuls off a memset tile so the PE
            # clock-gate reaches 8/8 before the real stream begins. Without
            # this the first ~3.4us of real matmuls run at 1.2 GHz.
            nc.vector.memset(warm_t[:], 1.0)
            warm_ps = psump.tile([128, 512], f32, tag="ps")
            for _ in range(36):
                nc.tensor.matmul(
                    out=warm_ps[:8, :8], lhsT=warm_t[:, :], rhs=warm_t[:, :],
                    start=True, stop=True,
                )

            # Inputs split over two DGE queues: x tiles stream on the sync
            # ring in demand order; W + bias go on the gpsimd ring so the
            # first superblock's weights don't queue behind x bulk data.
            nc.gpsimd.dma_start(out=wf_t[:], in_=wf_d[:, :])
            nc.gpsimd.dma_start(out=bias_t[:], in_=bias_d[:, :])
            for g in range(NCH):
                nc.gpsimd.dma_start(out=wb_t[:, g], in_=wb_d[:, g])
            for mh in range(WINDOW):
                _x_load(mh)

            for mh in range(N_MH):
                xf, xb = xfs[mh], xbs[mh]
                # Phase A: open all 8 groups with back-to-back DoubleRow MMs
                pss = []
                for g in range(NCH):
                    ps = psump.tile([128, 512], f32, tag="ps")
                    pss.append(ps)
                    for p in range(NDR):
                        nc.tensor.matmul(
                            out=ps[:],
                            lhsT=wf_t[:, g, p],
                            rhs=xf[:, p],
                            start=(p == 0),
                            stop=False,
                            perf_mode=DR,
                        )
                # Phase B: close each group with bf16 MMs, then bias + store.
                # Alternate ACT/DVE for the bias-add and their two DGE
                # queues for the store so the final drain runs 2x wide.
                for g in range(NCH):
                    ps = pss[g]
                    for kc in range(NB):
                        nc.tensor.matmul(
                            out=ps[:],
                            lhsT=wb_t[:, g, kc * 128:(kc + 1) * 128],
                            rhs=xb[:, kc],
                            start=False,
                            stop=(kc == NB - 1),
                        )
                    ot = outp.tile([128, 512], f32, tag="ot")
                    out_ap = out_d[g * 128:(g + 1) * 128,
                                   mh * 512:(mh + 1) * 512]
                    if g % 2 == 0:
                        nc.scalar.add(
                            out=ot[:], in_=ps[:], add=bias_t[:, g:g + 1]
                        )
                        nc.scalar.dma_start(out=out_ap, in_=ot[:])
                    else:
                        nc.vector.tensor_scalar_add(
                            ot[:], ps[:], bias_t[:, g:g + 1]
                        )
                        nc.gpsimd.dma_start(out=out_ap, in_=ot[:])
                # Prefetch the m-half that reuses this window slot
                if mh + WINDOW < N_MH:
                    _x_load(mh + WINDOW)
    nc.compile()
    return nc


def get_nc():
    if "nc" not in _CACHE:
        _CACHE["nc"] = _build()
    return _CACHE["nc"]


def make_in_maps(x, codebook, indices, bias):
    """Host-side sharding: full inputs -> per-core input dicts."""
    import ml_dtypes

    bf16 = ml_dtypes.bfloat16
    e4m3 = ml_dtypes.float8_e4m3  # TRN FP8_EXP4 variant (max 240)

    xm = np.asarray(x, dtype=np.float32).reshape(M_FULL, IN_F)
    # x5d[mh, m, kcall, p] = xm[mh*512+m, kcall*128+p]
    x5d = xm.reshape(N_MH, 512, KC, 128)
    # bf16 part: kc 4..15 -> xtb[p, mh, kc, m]
    xtb = np.ascontiguousarray(
        x5d[:, :, NF8:, :].transpose(3, 0, 2, 1)
    ).astype(bf16)
    # fp8 part: kc 0..3 -> xtf[p, mh, pr, s, m]
    xtf = np.ascontiguousarray(
        x5d[:, :, :NF8, :].reshape(N_MH, 512, NDR, 2, 128)
        .transpose(4, 0, 2, 3, 1)
    ).astype(e4m3)

    cbf = np.asarray(codebook, dtype=np.float32)
    idx_all = np.asarray(indices, dtype=np.int64)
    W = cbf[idx_all].reshape(IN_F, OUT_F)
    bias = np.asarray(bias, dtype=np.float32)

    in_maps = []
    for c in range(N_CORES):
        Wc = W[:, c * N_PER:(c + 1) * N_PER]
        # w4d[kcall, p, g, col] = Wc[kcall*128+p, g*128+col]
        w4d = Wc.reshape(KC, 128, NCH, 128)
        wb = np.ascontiguousarray(
            w4d[NF8:].transpose(1, 2, 0, 3)
        ).reshape(128, NCH, NB * 128).astype(bf16)
        wf = np.ascontiguousarray(
            w4d[:NF8].reshape(NDR, 2, 128, NCH, 128).transpose(2, 3, 0, 1, 4)
        ).astype(e4m3)
        bias_c = np.ascontiguousarray(
            bias[c * N_PER:(c + 1) * N_PER].reshape(NCH, 128).T
        )
        in_maps.append(
            {"xtb": xtb, "xtf": xtf, "wb": wb, "wf": wf, "biasT": bias_c}
        )
    return in_maps


def _spot_check(out2d, xm, W, bias):
    """Cheap integrity check: verify a random sample of outputs on the host.
    Healthy runs sit at sample rel err ~0.01 (fp8 split-K quantization);
    the threshold only trips on catastrophic corruption (a transient
    device flake was once observed on a fresh NEFF's first execution)."""
    rng = np.random.default_rng(0)
    mi = rng.integers(0, M_FULL, 256)
    ni = rng.integers(0, OUT_F, 256)
    ref = np.einsum("ij,ij->i", xm[mi], W[:, ni].T) + bias[ni]
    scale = max(np.abs(ref).max(), 1.0)
    dev = np.abs(out2d[mi, ni] - ref).max() / scale
    return float(dev)


def kernel(x, codebook, indices, bias):
    from concourse.bass_utils import run_bass_kernel_spmd

    nc = get_nc()
    in_maps = make_in_maps(x, codebook, indices, bias)

    xm = np.asarray(x, dtype=np.float32).reshape(M_FULL, IN_F)
    W = np.asarray(codebook, dtype=np.float32)[
        np.asarray(indices, dtype=np.int64)
    ].reshape(IN_F, OUT_F)
    bias_f = np.asarray(bias, dtype=np.float32)

    for _ in range(2):
        res = run_bass_kernel_spmd(nc, in_maps, core_ids=list(range(N_CORES)))
        # outT is [n, m] per core; stack along n then transpose to [m, n]
        full = np.concatenate(
            [np.asarray(res.results[c]["outT"], dtype=np.float32)
             for c in range(N_CORES)],
            axis=0,
        )
        out2d = np.ascontiguousarray(full.T)
        if _spot_check(out2d, xm, W, bias_f) < 0.1:
            break
    out = out2d.reshape(4, 2048, OUT_F)
    return out.astype(np.float32, copy=False)


# revision 11
# speedup vs baseline: 1.1446x; 1.0160x over previous
"""Trainium2 Bass kernel for CodebookConv1D (VQ-dequant + GPT2-Conv1D matmul).

Computation: W = codebook[indices].reshape(2048, 8192); out = x @ W + bias.
Sharding: tensor-parallel over out_features (8192 -> 8 cores x 1024 columns).

Per core (out columns split into 8 n-chunks of 128, tokens into 16 m-halves
of 512), a mixed-precision split-K scheme:
  - K = 2048 is split 4 chunks fp8-e4m3 + 12 chunks bf16. The fp8 chunks
    run as fp8 DoubleRow matmuls (2 k-chunks per MM, ~250 ns vs 2x216 ns
    for bf16) which cuts PE time ~10.5% while the quantization error stays
    at rel ~0.0185 < 2e-2 (error dilutes as sqrt(K_fp8/K); pure fp8 would
    be 0.038).
  - DoubleRow MMs pay a ~135 ns mode-switch penalty when adjacent to bf16
    MMs, so the (mh) superblock opens all 8 n-groups' accumulations with
    16 back-to-back DoubleRow MMs (phase A, one PSUM bank per n-chunk g),
    then closes them with 96 back-to-back bf16 MMs (phase B): 2 switches
    per 24.7 us instead of per group.
  - W is fully dequantized on the host (cb[idx]) and shipped pre-cast:
    wb (bf16 12/16 of rows) + wf (e4m3 4/16, DoubleRow pair layout). x is
    host-transposed/cast the same way. Total HBM in+out ~65 MB/core,
    well under the PE time at 358 GB/s.
  - Bias is added on the Activation engine (per-partition bias vector)
    while copying PSUM -> SBUF; output is stored n-on-partitions
    (transposed), the host transposes back when unsharding.
"""

import sys

if "/opt/trn_rl_repo" not in sys.path:
    sys.path.insert(0, "/opt/trn_rl_repo")

import numpy as np

IN_F = 2048
OUT_F = 8192
K_CB = 4096
BLOCK = 8
N_CORES = 8
M_FULL = 8192                      # 4*2048 tokens
N_PER = OUT_F // N_CORES           # 1024 out columns per core
KC = IN_F // 128                   # 16 k-chunks
NCH = N_PER // 128                 # 8 n-chunks of 128 columns per core
NF8 = 4                            # k-chunks in fp8 (kc 0..3), as 2 DR pairs
NDR = NF8 // 2                     # DoubleRow matmuls per group
NB = KC - NF8                      # bf16 k-chunks (kc 4..15)
N_MH = M_FULL // 512               # 16 m-halves of 512 tokens
WINDOW = 8                         # resident m-halves per window

_CACHE = {}


def _build():
    import concourse.bacc as bacc
    import concourse.mybir as mybir
    import concourse.tile as tile

    f32 = mybir.dt.float32
    bf16 = mybir.dt.bfloat16
    f8e4 = mybir.dt.float8e4
    DR = mybir.MatmulPerfMode.DoubleRow

    nc = bacc.Bacc("TRN2", target_bir_lowering=False)
    # Host-tiled x, bf16 part: xtb[p, mh, kc, m] = x[mh*512+m, (NF8+kc)*128+p]
    xtb_d = nc.dram_tensor("xtb", [128, N_MH, NB, 512], bf16,
                           kind="ExternalInput")
    # fp8 part in DoubleRow pair layout: xtf[p, mh, pr, s, m]
    xtf_d = nc.dram_tensor("xtf", [128, N_MH, NDR, 2, 512], f8e4,
                           kind="ExternalInput")
    # W shards, host-dequantized: wb[p, (g, kc, col)], wf[p, (g, pr, s, col)]
    wb_d = nc.dram_tensor("wb", [128, NCH, NB * 128], bf16,
                          kind="ExternalInput")
    wf_d = nc.dram_tensor("wf", [128, NCH, NDR, 2, 128], f8e4,
                          kind="ExternalInput")
    # biasT[p, g] = bias[g*128 + p]
    bias_d = nc.dram_tensor("biasT", [128, NCH], f32, kind="ExternalInput")
    # Output stored transposed: outT[n, m]
    out_d = nc.dram_tensor("outT", [N_PER, M_FULL], f32, kind="ExternalOutput")

    with tile.TileContext(nc) as tc:
        with (
            tc.tile_pool(name="const", bufs=1) as constp,
            tc.tile_pool(name="xio", bufs=WINDOW) as xio,
            tc.tile_pool(name="outp", bufs=4) as outp,
            tc.tile_pool(name="psum", bufs=8, space="PSUM") as psump,
        ):
            bias_t = constp.tile([128, NCH], f32)
            wf_t = constp.tile([128, NCH, NDR, 2, 128], f8e4)
            wb_t = constp.tile([128, NCH, NB * 128], bf16)
            warm_t = constp.tile([128, 512], bf16)

            xbs, xfs = {}, {}

            def _x_load(mh):
                xf = xio.tile([128, NDR, 2, 512], f8e4, tag="xf")
                nc.sync.dma_start(out=xf[:], in_=xtf_d[:, mh])
                xfs[mh] = xf
                xb = xio.tile([128, NB, 512], bf16, tag="xb")
                nc.sync.dma_start(out=xb[:], in_=xtb_d[:, mh])
                xbs[mh] = xb

            # HAM warmup: the DMA rings take ~8.7us to move the first input
            # byte. Fill that window with N=512 matmuls off a memset tile so
            # the PE clock-gate reaches 8/8 (and stays there) before the real
            # stream begins; otherwise its first ~3.4us run at 1.2 GHz.
            nc.vector.memset(warm_t[:], 1.0)
            warm_ps = psump.tile([128, 512], f32, tag="ps")
            for _ in range(30):
                nc.tensor.matmul(
                    out=warm_ps[:8, :],
                    lhsT=warm_t[:, :8],
                    rhs=warm_t[:, :],
                    start=True, stop=True,
                )

            # Demand-ordered startup on the sync ring: the first superblock
            # needs wf (all g) + xf0 for phase A, then wb-g in phase-B order
            # interleaved with the next m-halves' x tiles.
            nc.sync.dma_start(out=wf_t[:], in_=wf_d[:, :])
            nc.sync.dma_start(out=bias_t[:], in_=bias_d[:, :])
            _x_load(0)
            nc.sync.dma_start(out=wb_t[:, 0], in_=wb_d[:, 0])
            nc.sync.dma_start(out=wb_t[:, 1], in_=wb_d[:, 1])
            _x_load(1)
            for g in range(2, NCH):
                nc.sync.dma_start(out=wb_t[:, g], in_=wb_d[:, g])
            for mh in range(2, WINDOW):
                _x_load(mh)

            for mh in range(N_MH):
                xf, xb = xfs[mh], xbs[mh]
                # Phase A: open all 8 groups with back-to-back DoubleRow MMs
                pss = []
                for g in range(NCH):
                    ps = psump.tile([128, 512], f32, tag="ps")
                    pss.append(ps)
                    for p in range(NDR):
                        nc.tensor.matmul(
                            out=ps[:],
                            lhsT=wf_t[:, g, p],
                            rhs=xf[:, p],
                            start=(p == 0),
                            stop=False,
                            perf_mode=DR,
                        )
                # Phase B: close each group with bf16 MMs, then bias + store.
                # Alternate ACT/DVE for the bias-add and their two DGE
                # queues for the store so the final drain runs 2x wide.
                for g in range(NCH):
                    ps = pss[g]
                    for kc in range(NB):
                        nc.tensor.matmul(
                            out=ps[:],
                            lhsT=wb_t[:, g, kc * 128:(kc + 1) * 128],
                            rhs=xb[:, kc],
                            start=False,
                            stop=(kc == NB - 1),
                        )
                    ot = outp.tile([128, 512], f32, tag="ot")
                    out_ap = out_d[g * 128:(g + 1) * 128,
                                   mh * 512:(mh + 1) * 512]
                    if g % 2 == 0:
                        nc.scalar.add(
                            out=ot[:], in_=ps[:], add=bias_t[:, g:g + 1]
                        )
                        nc.scalar.dma_start(out=out_ap, in_=ot[:])
                    else:
                        nc.vector.tensor_scalar_add(
                            ot[:], ps[:], bias_t[:, g:g + 1]
                        )
                        nc.scalar.dma_start(out=out_ap, in_=ot[:])
                # Prefetch the m-half that reuses this window slot
                if mh + WINDOW < N_MH:
                    _x_load(mh + WINDOW)
    nc.compile()
    return nc


def get_nc():
    if "nc" not in _CACHE:
        _CACHE["nc"] = _build()
    return _CACHE["nc"]


def make_in_maps(x, codebook, indices, bias):
    """Host-side sharding: full inputs -> per-core input dicts."""
    import ml_dtypes

    bf16 = ml_dtypes.bfloat16
    e4m3 = ml_dtypes.float8_e4m3  # TRN FP8_EXP4 variant (max 240)

    xm = np.asarray(x, dtype=np.float32).reshape(M_FULL, IN_F)
    # x5d[mh, m, kcall, p] = xm[mh*512+m, kcall*128+p]
    x5d = xm.reshape(N_MH, 512, KC, 128)
    # bf16 part: kc 4..15 -> xtb[p, mh, kc, m]
    xtb = np.ascontiguousarray(
        x5d[:, :, NF8:, :].transpose(3, 0, 2, 1)
    ).astype(bf16)
    # fp8 part: kc 0..3 -> xtf[p, mh, pr, s, m]
    xtf = np.ascontiguousarray(
        x5d[:, :, :NF8, :].reshape(N_MH, 512, NDR, 2, 128)
        .transpose(4, 0, 2, 3, 1)
    ).astype(e4m3)

    cbf = np.asarray(codebook, dtype=np.float32)
    idx_all = np.asarray(indices, dtype=np.int64)
    W = cbf[idx_all].reshape(IN_F, OUT_F)
    bias = np.asarray(bias, dtype=np.float32)

    in_maps = []
    for c in range(N_CORES):
        Wc = W[:, c * N_PER:(c + 1) * N_PER]
        # w4d[kcall, p, g, col] = Wc[kcall*128+p, g*128+col]
        w4d = Wc.reshape(KC, 128, NCH, 128)
        wb = np.ascontiguousarray(
            w4d[NF8:].transpose(1, 2, 0, 3)
        ).reshape(128, NCH, NB * 128).astype(bf16)
        wf = np.ascontiguousarray(
            w4d[:NF8].reshape(NDR, 2, 128, NCH, 128).transpose(2, 3, 0, 1, 4)
        ).astype(e4m3)
        bias_c = np.ascontiguousarray(
            bias[c * N_PER:(c + 1) * N_PER].reshape(NCH, 128).T
        )
        in_maps.append(
            {"xtb": xtb, "xtf": xtf, "wb": wb, "wf": wf, "biasT": bias_c}
        )
    return in_maps


def _spot_check(out2d, xm, W, bias):
    """Cheap integrity check: verify a random sample of outputs on the host.
    Healthy runs sit at sample rel err ~0.01 (fp8 split-K quantization);
    the threshold only trips on catastrophic corruption (a transient
    device flake was once observed on a fresh NEFF's first execution)."""
    rng = np.random.default_rng(0)
    mi = rng.integers(0, M_FULL, 256)
    ni = rng.integers(0, OUT_F, 256)
    ref = np.einsum("ij,ij->i", xm[mi], W[:, ni].T) + bias[ni]
    scale = max(np.abs(ref).max(), 1.0)
    dev = np.abs(out2d[mi, ni] - ref).max() / scale
    return float(dev)


def kernel(x, codebook, indices, bias):
    from concourse.bass_utils import run_bass_kernel_spmd

    nc = get_nc()
    in_maps = make_in_maps(x, codebook, indices, bias)

    xm = np.asarray(x, dtype=np.float32).reshape(M_FULL, IN_F)
    W = np.asarray(codebook, dtype=np.float32)[
        np.asarray(indices, dtype=np.int64)
    ].reshape(IN_F, OUT_F)
    bias_f = np.asarray(bias, dtype=np.float32)

    for _ in range(2):
        res = run_bass_kernel_spmd(nc, in_maps, core_ids=list(range(N_CORES)))
        # outT is [n, m] per core; stack along n then transpose to [m, n]
        full = np.concatenate(
            [np.asarray(res.results[c]["outT"], dtype=np.float32)
             for c in range(N_CORES)],
            axis=0,
        )
        out2d = np.ascontiguousarray(full.T)
        if _spot_check(out2d, xm, W, bias_f) < 0.1:
            break
    out = out2d.reshape(4, 2048, OUT_F)
    return out.astype(np.float32, copy=False)
